# revision 49
# baseline (speedup 1.0000x reference)
"""Trainium2 Bass kernel for nn_LoRAPool (MoE top-2 LoRA expert pool).

Math (reference):
    gates[t,e] = p_L[t,e] if e in top-2 of p_L[t,:] else 0
    hr[t,e,r]  = sum_d h[t,d] * A[e,r,d]
    out[t,d]   = sum_{e,r} hr[t,e,r] * 2.0 * gates[t,e] * B[e,d,r]

Folded into two dense matmuls over c = (e,r) in [0,128):
    A_cat[d,c] = 2.0 * A[e,r,d];  B_cat[c,d] = B[e,d,r]
    U^T[c,t]   = sum_d A_cat[d,c] hT[d,t]       (stage 1, PE, bf16)
    Us[c,t]    = U^T[c,t] * G[c,t]              (gating, DVE)
    out[t,d]   = sum_c Us[c,t] B_cat[c,d]       (stage 2, PE, bf16)

Memory-bound: all large traffic (h in, out) is bf16 (tolerance 2e-2; bf16
end-to-end error is ~6e-3). h is pre-transposed AND pre-tiled on the host
([p, group, k, t] layout) so every device DMA has 8KB contiguous lines and
no on-device transposes are needed. The top-2 routing gates are computed
on the host (f32-exact selection, bf16 values) and streamed in compact
[8, tokens] form (32 KB/core); one tiny matmul per group expands them to
the dense mask G[c,t] = gates[t, c//16] during otherwise-idle PE time.
Token groups of 512 pipeline: group g's store overlaps group g+1's load.

Sharding: tokens (4*4096 = 16384) split evenly across 8 cores; weights
replicated.
"""

import numpy as np

N_CORES = 8
B_SZ, S_SZ, D = 4, 4096, 2048
E, R, C = 8, 16, 128
T_FULL = B_SZ * S_SZ            # 16384 tokens
T_CORE = T_FULL // N_CORES      # 2048 tokens per core
GROUP = 512                     # token group (stage-1 PSUM bank width)
N_GROUPS = T_CORE // GROUP      # 4
N_SUBTOT = T_CORE // 128        # 16 sub-blocks of 128 tokens per core
SUB_PER_GROUP = GROUP // 128    # 4
KD = D // 128                   # 16 contraction chunks
KH = KD // 2                    # chunks per hT half-DMA
SCALING = 2.0

_CACHE = {}


def _build_nc(split_waits=True):
    import concourse.bass as bass
    import concourse.tile as tile
    import concourse.mybir as mybir
    from contextlib import ExitStack

    f32 = mybir.dt.float32
    bf16 = mybir.dt.bfloat16

    nc = bass.Bass()
    # hT[p, g*KD*GROUP + k*GROUP + t] = h[token g*GROUP+t, d = k*128+p]
    ht_d = nc.declare_dram_parameter(
        "hT", [128, N_GROUPS * KD * GROUP], bf16, isOutput=False
    )
    gt_d = nc.declare_dram_parameter("gt8", [E, T_CORE], bf16, isOutput=False)
    m_d = nc.declare_dram_parameter("Mexp", [E, C], bf16, isOutput=False)
    a_d = nc.declare_dram_parameter("A_cat", [128, KD * C], bf16, isOutput=False)
    b_d = nc.declare_dram_parameter("B_cat", [C, D], bf16, isOutput=False)
    o_d = nc.declare_dram_parameter("out", [T_CORE, D], bf16, isOutput=True)

    OP = mybir.AluOpType

    with ExitStack() as ctx:
        tc = ctx.enter_context(tile.TileContext(nc))
        consts = ctx.enter_context(tc.tile_pool(name="consts", bufs=1))
        hpool = ctx.enter_context(tc.tile_pool(name="h", bufs=2 * N_GROUPS))
        gpool = ctx.enter_context(tc.tile_pool(name="gsb", bufs=N_GROUPS))
        utspool = ctx.enter_context(tc.tile_pool(name="uts", bufs=2))
        # one o_sb per sub-tile: slot reuse would make copies wait on the
        # store DMA queue counter (= ALL earlier DMAs incl. the h stream)
        opool = ctx.enter_context(tc.tile_pool(name="osb", bufs=N_SUBTOT))
        ps_u = ctx.enter_context(tc.tile_pool(name="ps_u", bufs=2, space="PSUM"))
        ps_o = ctx.enter_context(tc.tile_pool(name="ps_o", bufs=3, space="PSUM"))

        # ---- constants first (small, clears the queue) ----
        A_sb = consts.tile([128, KD * C], bf16)
        nc.sync.dma_start(out=A_sb, in_=a_d[:, :])
        gt_sb = consts.tile([E, T_CORE], bf16)
        nc.sync.dma_start(out=gt_sb, in_=gt_d[:, :])
        M_sb = consts.tile([E, C], bf16)
        nc.sync.dma_start(out=M_sb, in_=m_d[:, :])

        def issue_ht(g):
            tiles = []
            for h2 in range(2):
                ht = hpool.tile([128, KH, GROUP], bf16, tag="h", name=f"ht{g}_{h2}")
                off = g * KD * GROUP + h2 * KH * GROUP
                nc.sync.dma_start(
                    out=ht,
                    in_=ht_d[:, off : off + KH * GROUP].rearrange(
                        "p (k t) -> p k t", k=KH
                    ),
                )
                tiles.append(ht)
            return tiles

        # issue ALL h loads before any output store enters the (in-order)
        # sync queue — otherwise stores head-of-line block later h streams
        ht_tiles = {0: issue_ht(0)}
        B_sb = consts.tile([C, D], bf16)
        nc.sync.dma_start(out=B_sb, in_=b_d[:, :])
        for g in range(1, N_GROUPS):
            ht_tiles[g] = issue_ht(g)

        def stage1_chunk(g, U_ps, k0, k1):
            for k in range(k0, k1):
                nc.tensor.matmul(
                    U_ps,
                    lhsT=A_sb[:, k * C : (k + 1) * C],
                    rhs=ht_tiles[g][k // KH][:, k % KH, :],
                    start=(k == 0),
                    stop=(k == KD - 1),
                )

        def stage1(g):
            U_ps = ps_u.tile([128, GROUP], f32, tag="u", name=f"U{g}")
            stage1_chunk(g, U_ps, 0, KD)
            return U_ps

        copy_flip = [0]

        def stage2_subs(g, uts, subs):
            for s4 in subs:
                s = g * SUB_PER_GROUP + s4
                o_sb = opool.tile([128, D], bf16, tag="osb", name=f"osb{s}")
                for jh in range(2):
                    o_ps = ps_o.tile([128, 1024], f32, tag="o", name=f"o{s}_{jh}")
                    for j2 in range(2):
                        j = jh * 2 + j2
                        nc.tensor.matmul(
                            o_ps[:, j2 * 512 : (j2 + 1) * 512],
                            lhsT=uts[:, s4 * 128 : (s4 + 1) * 128],
                            rhs=B_sb[:, j * 512 : (j + 1) * 512],
                            start=True,
                            stop=True,
                        )
                    if s == N_SUBTOT - 1:
                        # final sub-tile is the exposed tail: split each
                        # copy across both engines to halve its latency
                        nc.vector.tensor_copy(
                            out=o_sb[:, jh * 1024 : jh * 1024 + 512],
                            in_=o_ps[:, :512],
                        )
                        nc.scalar.copy(
                            out=o_sb[:, jh * 1024 + 512 : (jh + 1) * 1024],
                            in_=o_ps[:, 512:],
                        )
                    elif copy_flip[0] % 2 == 0:
                        nc.vector.tensor_copy(
                            out=o_sb[:, jh * 1024 : (jh + 1) * 1024], in_=o_ps
                        )
                    else:
                        nc.scalar.copy(
                            out=o_sb[:, jh * 1024 : (jh + 1) * 1024], in_=o_ps
                        )
                    copy_flip[0] += 1
                    # store each half right after its copy: halves the
                    # copy->store latency on every group-boundary chain
                    nc.sync.dma_start(
                        out=o_d[s * 128 : (s + 1) * 128, jh * 1024 : (jh + 1) * 1024],
                        in_=o_sb[:, jh * 1024 : (jh + 1) * 1024],
                    )

        # expand gt[8, t] -> dense G[c, t] per group up front. Expansion
        # tiles live in ps_u (same shape as U, no effect on the out-pool
        # slot rotation); copies go to the early-idle DVE.
        G_list = []
        for g in range(N_GROUPS):
            G_ps = ps_u.tile([128, GROUP], f32, tag="u", name=f"Gps{g}")
            nc.tensor.matmul(
                G_ps,
                lhsT=M_sb,
                rhs=gt_sb[:, g * GROUP : (g + 1) * GROUP],
                start=True,
                stop=True,
            )
            G_sbg = gpool.tile([128, GROUP], bf16, tag="gsb", name=f"Gsb{g}")
            nc.vector.tensor_copy(out=G_sbg, in_=G_ps)
            G_list.append(G_sbg)

        # Monotone logical waits stop the scheduler from hoisting group g+1
        # work above group g's store pipeline. Stage-1 of group g+1 is
        # emitted in 4-matmul chunks between group g's stage-2 sub-tiles so
        # scheduling mispredictions cost at most one small chunk.
        U_cur = stage1(0)
        for g in range(N_GROUPS):
            tc.tile_set_cur_wait(g + 1)
            uts = utspool.tile([128, GROUP], bf16, tag="uts", name=f"uts{g}")
            nc.vector.tensor_tensor(
                out=uts, in0=U_cur, in1=G_list[g], op=OP.mult
            )
            U_next = None
            if g + 1 < N_GROUPS:
                U_next = ps_u.tile([128, GROUP], f32, tag="u", name=f"U{g + 1}")
            for s4 in range(SUB_PER_GROUP):
                stage2_subs(g, uts, (s4,))
                if U_next is not None:
                    stage1_chunk(g + 1, U_next, 4 * s4, 4 * s4 + 4)
            U_cur = U_next

    if split_waits:
        _split_matmul_waits(nc)
    return nc


def _split_matmul_waits(nc, max_waits=1):
    """Walrus codegen allows only one sync-wait slot on self-loading
    Matmult instructions. Move surplus waits onto a no-op EventSemaphore
    inserted immediately before, same engine — identical semantics."""
    import concourse.mybir as mybir

    n = 0
    for f in nc.m.functions:
        for blk in f.blocks:
            insts = blk.instructions
            new_list = []
            changed = False
            for inst in insts:
                si = inst.sync_info
                if (
                    type(inst).__name__ != "InstEventSemaphore"
                    and si is not None
                    and si.on_wait
                    and len(si.on_wait) > max_waits
                ):
                    surplus = list(si.on_wait[:-max_waits])
                    keep = list(si.on_wait[-max_waits:])
                    for i in range(0, len(surplus), 2):
                        n += 1
                        ev = mybir.InstEventSemaphore(
                            name=f"I-swsplit-{n}", ins=[], outs=[]
                        )
                        ev.engine = inst.engine
                        ev.sync_info = mybir.SyncInfo(
                            on_wait=surplus[i : i + 2], on_update=[]
                        )
                        new_list.append(ev)
                    inst.sync_info = mybir.SyncInfo(
                        on_wait=keep, on_update=list(si.on_update or [])
                    )
                    changed = True
                new_list.append(inst)
            if changed:
                blk.instructions = new_list
    return n


def _host_prep(h, p_L, A, B):
    """Shard tokens across cores; pre-transpose + pre-tile h; compute the
    top-2 gate matrix G on the host."""
    import ml_dtypes

    BF16 = ml_dtypes.bfloat16

    # hT[core][p, g, k, t] = h[core][token g*GROUP+t, d = k*128+p]
    h5 = np.asarray(h, dtype=np.float32).reshape(N_CORES, N_GROUPS, GROUP, KD, 128)
    hT = np.ascontiguousarray(h5.transpose(0, 4, 1, 3, 2)).astype(BF16)
    hT = hT.reshape(N_CORES, 128, N_GROUPS * KD * GROUP)

    # top-2 gates, f32-exact selection (matches jax.lax.top_k on distinct
    # values); G[core][c, t] = gates[t, c//16]
    p_flat = np.asarray(p_L, dtype=np.float32).reshape(T_FULL, E)
    thr = np.partition(p_flat, E - 2, axis=1)[:, E - 2 : E - 1]  # 2nd largest
    gates = np.where(p_flat >= thr, p_flat, np.float32(0.0))
    gt8 = gates.T.astype(BF16)  # [E, T_FULL]
    gt8 = np.ascontiguousarray(gt8.reshape(E, N_CORES, T_CORE).transpose(1, 0, 2))
    Mexp = np.zeros((E, C), dtype=np.float32)
    for e in range(E):
        Mexp[e, e * R : (e + 1) * R] = 1.0
    Mexp = Mexp.astype(BF16)

    # A_cat[d, c] = SCALING * A[e, r, d], pre-arranged [p, k*C + c]
    A_cat = (np.asarray(A, dtype=np.float32) * SCALING).transpose(2, 0, 1).reshape(D, C)
    A_arr = np.ascontiguousarray(
        A_cat.reshape(KD, 128, C).transpose(1, 0, 2).reshape(128, KD * C)
    ).astype(BF16)
    # B_cat[c, d] = B[e, d, r]
    B_cat = (
        np.asarray(B, dtype=np.float32).transpose(0, 2, 1).reshape(C, D).astype(BF16)
    )

    in_maps = []
    for i in range(N_CORES):
        in_maps.append(
            {
                "hT": hT[i],
                "gt8": gt8[i],
                "Mexp": Mexp,
                "A_cat": A_arr,
                "B_cat": B_cat,
            }
        )
    return in_maps


def _get_nc():
    if "nc" not in _CACHE:
        _CACHE["nc"] = _build_nc()
    return _CACHE["nc"]


def kernel(h, p_L, A, B):
    from concourse.bass_utils import run_bass_kernel_spmd

    nc = _get_nc()
    in_maps = _host_prep(h, p_L, A, B)
    res = run_bass_kernel_spmd(nc, in_maps, core_ids=list(range(N_CORES)))
    out = np.concatenate(
        [np.asarray(res.results[i]["out"]) for i in range(N_CORES)], axis=0
    )
    return out.astype(np.float32).reshape(B_SZ, S_SZ, D)


# revision 52
# speedup vs baseline: 1.1161x; 1.1161x over previous
"""Trainium2 Bass kernel for nn_LoRAPool (MoE top-2 LoRA expert pool).

Math (reference):
    gates[t,e] = p_L[t,e] if e in top-2 of p_L[t,:] else 0
    hr[t,e,r]  = sum_d h[t,d] * A[e,r,d]
    out[t,d]   = sum_{e,r} hr[t,e,r] * 2.0 * gates[t,e] * B[e,d,r]

Folded into two dense matmuls over c = (e,r) in [0,128):
    A_cat[d,c] = 2.0 * A[e,r,d];  B_cat[c,d] = B[e,d,r]
    U^T[c,t]   = sum_d A_cat[d,c] hT[d,t]       (stage 1, PE, bf16)
    Us[c,t]    = U^T[c,t] * G[c,t]              (gating, DVE)
    out[t,d]   = sum_c Us[c,t] B_cat[c,d]       (stage 2, PE, bf16)

Memory-bound: all large traffic (h in, out) is bf16 (tolerance 2e-2; bf16
end-to-end error is ~6e-3). h is pre-transposed AND pre-tiled on the host
([p, group, k, t] layout) so every device DMA has 8KB contiguous lines and
no on-device transposes are needed. The top-2 routing gates are computed
on the host (f32-exact selection, bf16 values) and streamed in compact
[8, tokens] form (32 KB/core); one tiny matmul per group expands them to
the dense mask G[c,t] = gates[t, c//16] during otherwise-idle PE time.
Token groups of 512 pipeline: group g's store overlaps group g+1's load.

Sharding: tokens (4*4096 = 16384) split evenly across 8 cores; weights
replicated.
"""

import numpy as np

N_CORES = 8
B_SZ, S_SZ, D = 4, 4096, 2048
E, R, C = 8, 16, 128
T_FULL = B_SZ * S_SZ            # 16384 tokens
T_CORE = T_FULL // N_CORES      # 2048 tokens per core
GROUP = 512                     # token group (stage-1 PSUM bank width)
N_GROUPS = T_CORE // GROUP      # 4
N_SUBTOT = T_CORE // 128        # 16 sub-blocks of 128 tokens per core
SUB_PER_GROUP = GROUP // 128    # 4
KD = D // 128                   # 16 contraction chunks
KH = KD // 2                    # chunks per hT half-DMA
SCALING = 2.0

_CACHE = {}


def _build_nc(split_waits=True):
    import concourse.bass as bass
    import concourse.tile as tile
    import concourse.mybir as mybir
    from contextlib import ExitStack

    f32 = mybir.dt.float32
    bf16 = mybir.dt.bfloat16

    nc = bass.Bass()
    # hT[p, g*KD*GROUP + k*GROUP + t] = h[token g*GROUP+t, d = k*128+p]
    ht_d = nc.declare_dram_parameter(
        "hT", [128, N_GROUPS * KD * GROUP], bf16, isOutput=False
    )
    gt_d = nc.declare_dram_parameter("gt8", [E, T_CORE], bf16, isOutput=False)
    m_d = nc.declare_dram_parameter("Mexp", [E, C], bf16, isOutput=False)
    a_d = nc.declare_dram_parameter("A_cat", [128, KD * C], bf16, isOutput=False)
    b_d = nc.declare_dram_parameter("B_cat", [C, D], bf16, isOutput=False)
    o_d = nc.declare_dram_parameter("out", [T_CORE, D], bf16, isOutput=True)

    OP = mybir.AluOpType

    with ExitStack() as ctx:
        tc = ctx.enter_context(tile.TileContext(nc))
        consts = ctx.enter_context(tc.tile_pool(name="consts", bufs=1))
        hpool = ctx.enter_context(tc.tile_pool(name="h", bufs=2 * N_GROUPS))
        gpool = ctx.enter_context(tc.tile_pool(name="gsb", bufs=N_GROUPS))
        utspool = ctx.enter_context(tc.tile_pool(name="uts", bufs=2))
        # one o_sb per sub-tile: slot reuse would make copies wait on the
        # store DMA queue counter (= ALL earlier DMAs incl. the h stream)
        opool = ctx.enter_context(tc.tile_pool(name="osb", bufs=N_SUBTOT))
        ps_u = ctx.enter_context(tc.tile_pool(name="ps_u", bufs=2, space="PSUM"))
        ps_o = ctx.enter_context(tc.tile_pool(name="ps_o", bufs=3, space="PSUM"))

        # ---- A first (needed by the first stage-1 matmul) ----
        A_sb = consts.tile([128, KD * C], bf16)
        nc.sync.dma_start(out=A_sb, in_=a_d[:, :])

        def issue_ht(g):
            tiles = []
            for h2 in range(2):
                ht = hpool.tile([128, KH, GROUP], bf16, tag="h", name=f"ht{g}_{h2}")
                off = g * KD * GROUP + h2 * KH * GROUP
                nc.sync.dma_start(
                    out=ht,
                    in_=ht_d[:, off : off + KH * GROUP].rearrange(
                        "p (k t) -> p k t", k=KH
                    ),
                )
                tiles.append(ht)
            return tiles

        # issue ALL h loads before any output store enters the (in-order)
        # sync queue — otherwise stores head-of-line block later h streams.
        # gt/Mexp/B slot in behind group 0's h: they're small, not needed
        # until ~16us, and ahead of the h stream they would delay it.
        ht_tiles = {0: issue_ht(0)}
        gt_sb = consts.tile([E, T_CORE], bf16)
        nc.sync.dma_start(out=gt_sb, in_=gt_d[:, :])
        M_sb = consts.tile([E, C], bf16)
        nc.sync.dma_start(out=M_sb, in_=m_d[:, :])
        B_sb = consts.tile([C, D], bf16)
        nc.sync.dma_start(out=B_sb, in_=b_d[:, :])
        for g in range(1, N_GROUPS):
            ht_tiles[g] = issue_ht(g)

        def stage1_chunk(g, U_ps, k0, k1):
            for k in range(k0, k1):
                nc.tensor.matmul(
                    U_ps,
                    lhsT=A_sb[:, k * C : (k + 1) * C],
                    rhs=ht_tiles[g][k // KH][:, k % KH, :],
                    start=(k == 0),
                    stop=(k == KD - 1),
                )

        def stage1(g):
            U_ps = ps_u.tile([128, GROUP], f32, tag="u", name=f"U{g}")
            stage1_chunk(g, U_ps, 0, KD)
            return U_ps

        copy_flip = [0]

        def stage2_subs(g, uts, subs):
            for s4 in subs:
                s = g * SUB_PER_GROUP + s4
                o_sb = opool.tile([128, D], bf16, tag="osb", name=f"osb{s}")
                for jh in range(2):
                    o_ps = ps_o.tile([128, 1024], f32, tag="o", name=f"o{s}_{jh}")
                    for j2 in range(2):
                        j = jh * 2 + j2
                        nc.tensor.matmul(
                            o_ps[:, j2 * 512 : (j2 + 1) * 512],
                            lhsT=uts[:, s4 * 128 : (s4 + 1) * 128],
                            rhs=B_sb[:, j * 512 : (j + 1) * 512],
                            start=True,
                            stop=True,
                        )
                    if s == N_SUBTOT - 1:
                        # final sub-tile is the exposed tail: split each
                        # copy across both engines to halve its latency
                        nc.vector.tensor_copy(
                            out=o_sb[:, jh * 1024 : jh * 1024 + 512],
                            in_=o_ps[:, :512],
                        )
                        nc.scalar.copy(
                            out=o_sb[:, jh * 1024 + 512 : (jh + 1) * 1024],
                            in_=o_ps[:, 512:],
                        )
                    elif copy_flip[0] % 2 == 0:
                        nc.vector.tensor_copy(
                            out=o_sb[:, jh * 1024 : (jh + 1) * 1024], in_=o_ps
                        )
                    else:
                        nc.scalar.copy(
                            out=o_sb[:, jh * 1024 : (jh + 1) * 1024], in_=o_ps
                        )
                    copy_flip[0] += 1
                    # store each half right after its copy: halves the
                    # copy->store latency on every group-boundary chain
                    nc.sync.dma_start(
                        out=o_d[s * 128 : (s + 1) * 128, jh * 1024 : (jh + 1) * 1024],
                        in_=o_sb[:, jh * 1024 : (jh + 1) * 1024],
                    )

        # Monotone logical waits stop the scheduler from hoisting group g+1
        # work above group g's store pipeline. Stage-1 of group g+1 is
        # emitted in 4-matmul chunks between group g's stage-2 sub-tiles so
        # scheduling mispredictions cost at most one small chunk.
        U_cur = stage1(0)

        # expand gt[8, t] -> dense G[c, t] per group (after stage-1 g0 in
        # the PE queue: gt arrives mid-h-stream and must not block it).
        # Expansion tiles live in ps_u (same shape as U, no effect on the
        # out-pool slot rotation); copies go to the early-idle DVE.
        G_list = []
        for g in range(N_GROUPS):
            G_ps = ps_u.tile([128, GROUP], f32, tag="u", name=f"Gps{g}")
            nc.tensor.matmul(
                G_ps,
                lhsT=M_sb,
                rhs=gt_sb[:, g * GROUP : (g + 1) * GROUP],
                start=True,
                stop=True,
            )
            G_sbg = gpool.tile([128, GROUP], bf16, tag="gsb", name=f"Gsb{g}")
            nc.vector.tensor_copy(out=G_sbg, in_=G_ps)
            G_list.append(G_sbg)
        for g in range(N_GROUPS):
            tc.tile_set_cur_wait(g + 1)
            uts = utspool.tile([128, GROUP], bf16, tag="uts", name=f"uts{g}")
            nc.vector.tensor_tensor(
                out=uts, in0=U_cur, in1=G_list[g], op=OP.mult
            )
            U_next = None
            if g + 1 < N_GROUPS:
                U_next = ps_u.tile([128, GROUP], f32, tag="u", name=f"U{g + 1}")
            for s4 in range(SUB_PER_GROUP):
                stage2_subs(g, uts, (s4,))
                if U_next is not None:
                    stage1_chunk(g + 1, U_next, 4 * s4, 4 * s4 + 4)
            U_cur = U_next

    if split_waits:
        _split_matmul_waits(nc)
    return nc


def _split_matmul_waits(nc, max_waits=1):
    """Walrus codegen allows only one sync-wait slot on self-loading
    Matmult instructions. Move surplus waits onto a no-op EventSemaphore
    inserted immediately before, same engine — identical semantics."""
    import concourse.mybir as mybir

    n = 0
    for f in nc.m.functions:
        for blk in f.blocks:
            insts = blk.instructions
            new_list = []
            changed = False
            for inst in insts:
                si = inst.sync_info
                if (
                    type(inst).__name__ != "InstEventSemaphore"
                    and si is not None
                    and si.on_wait
                    and len(si.on_wait) > max_waits
                ):
                    surplus = list(si.on_wait[:-max_waits])
                    keep = list(si.on_wait[-max_waits:])
                    for i in range(0, len(surplus), 2):
                        n += 1
                        ev = mybir.InstEventSemaphore(
                            name=f"I-swsplit-{n}", ins=[], outs=[]
                        )
                        ev.engine = inst.engine
                        ev.sync_info = mybir.SyncInfo(
                            on_wait=surplus[i : i + 2], on_update=[]
                        )
                        new_list.append(ev)
                    inst.sync_info = mybir.SyncInfo(
                        on_wait=keep, on_update=list(si.on_update or [])
                    )
                    changed = True
                new_list.append(inst)
            if changed:
                blk.instructions = new_list
    return n


def _host_prep(h, p_L, A, B):
    """Shard tokens across cores; pre-transpose + pre-tile h; compute the
    top-2 gate matrix G on the host."""
    import ml_dtypes

    BF16 = ml_dtypes.bfloat16

    # hT[core][p, g, k, t] = h[core][token g*GROUP+t, d = k*128+p]
    h5 = np.asarray(h, dtype=np.float32).reshape(N_CORES, N_GROUPS, GROUP, KD, 128)
    hT = np.ascontiguousarray(h5.transpose(0, 4, 1, 3, 2)).astype(BF16)
    hT = hT.reshape(N_CORES, 128, N_GROUPS * KD * GROUP)

    # top-2 gates, f32-exact selection (matches jax.lax.top_k on distinct
    # values); G[core][c, t] = gates[t, c//16]
    p_flat = np.asarray(p_L, dtype=np.float32).reshape(T_FULL, E)
    thr = np.partition(p_flat, E - 2, axis=1)[:, E - 2 : E - 1]  # 2nd largest
    gates = np.where(p_flat >= thr, p_flat, np.float32(0.0))
    gt8 = gates.T.astype(BF16)  # [E, T_FULL]
    gt8 = np.ascontiguousarray(gt8.reshape(E, N_CORES, T_CORE).transpose(1, 0, 2))
    Mexp = np.zeros((E, C), dtype=np.float32)
    for e in range(E):
        Mexp[e, e * R : (e + 1) * R] = 1.0
    Mexp = Mexp.astype(BF16)

    # A_cat[d, c] = SCALING * A[e, r, d], pre-arranged [p, k*C + c]
    A_cat = (np.asarray(A, dtype=np.float32) * SCALING).transpose(2, 0, 1).reshape(D, C)
    A_arr = np.ascontiguousarray(
        A_cat.reshape(KD, 128, C).transpose(1, 0, 2).reshape(128, KD * C)
    ).astype(BF16)
    # B_cat[c, d] = B[e, d, r]
    B_cat = (
        np.asarray(B, dtype=np.float32).transpose(0, 2, 1).reshape(C, D).astype(BF16)
    )

    in_maps = []
    for i in range(N_CORES):
        in_maps.append(
            {
                "hT": hT[i],
                "gt8": gt8[i],
                "Mexp": Mexp,
                "A_cat": A_arr,
                "B_cat": B_cat,
            }
        )
    return in_maps


def _get_nc():
    if "nc" not in _CACHE:
        _CACHE["nc"] = _build_nc()
    return _CACHE["nc"]


def kernel(h, p_L, A, B):
    from concourse.bass_utils import run_bass_kernel_spmd

    nc = _get_nc()
    in_maps = _host_prep(h, p_L, A, B)
    res = run_bass_kernel_spmd(nc, in_maps, core_ids=list(range(N_CORES)))
    out = np.concatenate(
        [np.asarray(res.results[i]["out"]) for i in range(N_CORES)], axis=0
    )
    return out.astype(np.float32).reshape(B_SZ, S_SZ, D)


# revision 53
# speedup vs baseline: 1.1355x; 1.0173x over previous
"""Trainium2 Bass kernel for nn_LoRAPool (MoE top-2 LoRA expert pool).

Math (reference):
    gates[t,e] = p_L[t,e] if e in top-2 of p_L[t,:] else 0
    hr[t,e,r]  = sum_d h[t,d] * A[e,r,d]
    out[t,d]   = sum_{e,r} hr[t,e,r] * 2.0 * gates[t,e] * B[e,d,r]

Folded into two dense matmuls over c = (e,r) in [0,128):
    A_cat[d,c] = 2.0 * A[e,r,d];  B_cat[c,d] = B[e,d,r]
    U^T[c,t]   = sum_d A_cat[d,c] hT[d,t]       (stage 1, PE, bf16)
    Us[c,t]    = U^T[c,t] * G[c,t]              (gating, DVE)
    out[t,d]   = sum_c Us[c,t] B_cat[c,d]       (stage 2, PE, bf16)

Memory-bound: all large traffic (h in, out) is bf16 (tolerance 2e-2; bf16
end-to-end error is ~6e-3). h is pre-transposed AND pre-tiled on the host
([p, group, k, t] layout) so every device DMA has 8KB contiguous lines and
no on-device transposes are needed. The top-2 routing gates are computed
on the host (f32-exact selection, bf16 values) and streamed in compact
[8, tokens] form (32 KB/core); one tiny matmul per group expands them to
the dense mask G[c,t] = gates[t, c//16] during otherwise-idle PE time.
Token groups of 512 pipeline: group g's store overlaps group g+1's load.

Sharding: tokens (4*4096 = 16384) split evenly across 8 cores; weights
replicated.
"""

import numpy as np

N_CORES = 8
B_SZ, S_SZ, D = 4, 4096, 2048
E, R, C = 8, 16, 128
T_FULL = B_SZ * S_SZ            # 16384 tokens
T_CORE = T_FULL // N_CORES      # 2048 tokens per core
GROUP = 512                     # token group (stage-1 PSUM bank width)
N_GROUPS = T_CORE // GROUP      # 4
N_SUBTOT = T_CORE // 128        # 16 sub-blocks of 128 tokens per core
SUB_PER_GROUP = GROUP // 128    # 4
KD = D // 128                   # 16 contraction chunks
KH = KD // 2                    # chunks per hT half-DMA
SCALING = 2.0

_CACHE = {}


def _build_nc(split_waits=True):
    import concourse.bass as bass
    import concourse.tile as tile
    import concourse.mybir as mybir
    from contextlib import ExitStack

    f32 = mybir.dt.float32
    bf16 = mybir.dt.bfloat16

    nc = bass.Bass()
    # hT[p, g*KD*GROUP + k*GROUP + t] = h[token g*GROUP+t, d = k*128+p]
    ht_d = nc.declare_dram_parameter(
        "hT", [128, N_GROUPS * KD * GROUP], bf16, isOutput=False
    )
    gt_d = nc.declare_dram_parameter("gt8", [E, T_CORE], bf16, isOutput=False)
    m_d = nc.declare_dram_parameter("Mexp", [E, C], bf16, isOutput=False)
    a_d = nc.declare_dram_parameter("A_cat", [128, KD * C], bf16, isOutput=False)
    b_d = nc.declare_dram_parameter("B_cat", [C, D], bf16, isOutput=False)
    o_d = nc.declare_dram_parameter("out", [T_CORE, D], bf16, isOutput=True)

    OP = mybir.AluOpType

    with ExitStack() as ctx:
        tc = ctx.enter_context(tile.TileContext(nc))
        consts = ctx.enter_context(tc.tile_pool(name="consts", bufs=1))
        hpool = ctx.enter_context(tc.tile_pool(name="h", bufs=2 * N_GROUPS))
        gpool = ctx.enter_context(tc.tile_pool(name="gsb", bufs=N_GROUPS))
        utspool = ctx.enter_context(tc.tile_pool(name="uts", bufs=2))
        # one o_sb per sub-tile: slot reuse would make copies wait on the
        # store DMA queue counter (= ALL earlier DMAs incl. the h stream)
        opool = ctx.enter_context(tc.tile_pool(name="osb", bufs=N_SUBTOT))
        ps_u = ctx.enter_context(tc.tile_pool(name="ps_u", bufs=2, space="PSUM"))
        ps_o = ctx.enter_context(tc.tile_pool(name="ps_o", bufs=3, space="PSUM"))

        # ---- constants first (small, clears the queue) ----
        A_sb = consts.tile([128, KD * C], bf16)
        nc.sync.dma_start(out=A_sb, in_=a_d[:, :])
        gt_sb = consts.tile([E, T_CORE], bf16)
        nc.sync.dma_start(out=gt_sb, in_=gt_d[:, :])
        M_sb = consts.tile([E, C], bf16)
        nc.sync.dma_start(out=M_sb, in_=m_d[:, :])

        def issue_ht(g):
            tiles = []
            for h2 in range(2):
                ht = hpool.tile([128, KH, GROUP], bf16, tag="h", name=f"ht{g}_{h2}")
                off = g * KD * GROUP + h2 * KH * GROUP
                nc.sync.dma_start(
                    out=ht,
                    in_=ht_d[:, off : off + KH * GROUP].rearrange(
                        "p (k t) -> p k t", k=KH
                    ),
                )
                tiles.append(ht)
            return tiles

        # issue ALL h loads before any output store enters the (in-order)
        # sync queue — otherwise stores head-of-line block later h streams
        ht_tiles = {0: issue_ht(0)}
        B_sb = consts.tile([C, D], bf16)
        nc.sync.dma_start(out=B_sb, in_=b_d[:, :])
        for g in range(1, N_GROUPS):
            ht_tiles[g] = issue_ht(g)

        def stage1_chunk(g, U_ps, k0, k1):
            for k in range(k0, k1):
                nc.tensor.matmul(
                    U_ps,
                    lhsT=A_sb[:, k * C : (k + 1) * C],
                    rhs=ht_tiles[g][k // KH][:, k % KH, :],
                    start=(k == 0),
                    stop=(k == KD - 1),
                )

        def stage1(g):
            U_ps = ps_u.tile([128, GROUP], f32, tag="u", name=f"U{g}")
            stage1_chunk(g, U_ps, 0, KD)
            return U_ps

        copy_flip = [0]

        def stage2_subs(g, uts, subs):
            for s4 in subs:
                s = g * SUB_PER_GROUP + s4
                o_sb = opool.tile([128, D], bf16, tag="osb", name=f"osb{s}")
                for jh in range(2):
                    o_ps = ps_o.tile([128, 1024], f32, tag="o", name=f"o{s}_{jh}")
                    for j2 in range(2):
                        j = jh * 2 + j2
                        nc.tensor.matmul(
                            o_ps[:, j2 * 512 : (j2 + 1) * 512],
                            lhsT=uts[:, s4 * 128 : (s4 + 1) * 128],
                            rhs=B_sb[:, j * 512 : (j + 1) * 512],
                            start=True,
                            stop=True,
                        )
                    if s == N_SUBTOT - 1:
                        # final sub-tile is the exposed tail: split each
                        # copy across both engines to halve its latency
                        nc.vector.tensor_copy(
                            out=o_sb[:, jh * 1024 : jh * 1024 + 512],
                            in_=o_ps[:, :512],
                        )
                        nc.scalar.copy(
                            out=o_sb[:, jh * 1024 + 512 : (jh + 1) * 1024],
                            in_=o_ps[:, 512:],
                        )
                    elif copy_flip[0] % 2 == 0:
                        nc.vector.tensor_copy(
                            out=o_sb[:, jh * 1024 : (jh + 1) * 1024], in_=o_ps
                        )
                    else:
                        nc.scalar.copy(
                            out=o_sb[:, jh * 1024 : (jh + 1) * 1024], in_=o_ps
                        )
                    copy_flip[0] += 1
                    # store each half right after its copy: halves the
                    # copy->store latency on every group-boundary chain
                    nc.sync.dma_start(
                        out=o_d[s * 128 : (s + 1) * 128, jh * 1024 : (jh + 1) * 1024],
                        in_=o_sb[:, jh * 1024 : (jh + 1) * 1024],
                    )

        # expand gt[8, t] -> dense G[c, t] per group up front. Expansion
        # tiles live in ps_u (same shape as U, no effect on the out-pool
        # slot rotation); copies go to the early-idle DVE.
        G_list = []
        for g in range(N_GROUPS):
            G_ps = ps_u.tile([128, GROUP], f32, tag="u", name=f"Gps{g}")
            nc.tensor.matmul(
                G_ps,
                lhsT=M_sb,
                rhs=gt_sb[:, g * GROUP : (g + 1) * GROUP],
                start=True,
                stop=True,
            )
            G_sbg = gpool.tile([128, GROUP], bf16, tag="gsb", name=f"Gsb{g}")
            nc.vector.tensor_copy(out=G_sbg, in_=G_ps)
            G_list.append(G_sbg)

        # Monotone logical waits stop the scheduler from hoisting group g+1
        # work above group g's store pipeline. Stage-1 of group g+1 is
        # emitted in 4-matmul chunks between group g's stage-2 sub-tiles so
        # scheduling mispredictions cost at most one small chunk.
        U_cur = stage1(0)
        for g in range(N_GROUPS):
            tc.tile_set_cur_wait(g + 1)
            uts = utspool.tile([128, GROUP], bf16, tag="uts", name=f"uts{g}")
            nc.vector.tensor_tensor(
                out=uts, in0=U_cur, in1=G_list[g], op=OP.mult
            )
            U_next = None
            if g + 1 < N_GROUPS:
                U_next = ps_u.tile([128, GROUP], f32, tag="u", name=f"U{g + 1}")
            for s4 in range(SUB_PER_GROUP):
                stage2_subs(g, uts, (s4,))
                if U_next is not None:
                    stage1_chunk(g + 1, U_next, 4 * s4, 4 * s4 + 4)
            U_cur = U_next

    if split_waits:
        _split_matmul_waits(nc)
    return nc


def _split_matmul_waits(nc, max_waits=1):
    """Walrus codegen allows only one sync-wait slot on self-loading
    Matmult instructions. Move surplus waits onto a no-op EventSemaphore
    inserted immediately before, same engine — identical semantics."""
    import concourse.mybir as mybir

    n = 0
    for f in nc.m.functions:
        for blk in f.blocks:
            insts = blk.instructions
            new_list = []
            changed = False
            for inst in insts:
                si = inst.sync_info
                if (
                    type(inst).__name__ != "InstEventSemaphore"
                    and si is not None
                    and si.on_wait
                    and len(si.on_wait) > max_waits
                ):
                    surplus = list(si.on_wait[:-max_waits])
                    keep = list(si.on_wait[-max_waits:])
                    for i in range(0, len(surplus), 2):
                        n += 1
                        ev = mybir.InstEventSemaphore(
                            name=f"I-swsplit-{n}", ins=[], outs=[]
                        )
                        ev.engine = inst.engine
                        ev.sync_info = mybir.SyncInfo(
                            on_wait=surplus[i : i + 2], on_update=[]
                        )
                        new_list.append(ev)
                    inst.sync_info = mybir.SyncInfo(
                        on_wait=keep, on_update=list(si.on_update or [])
                    )
                    changed = True
                new_list.append(inst)
            if changed:
                blk.instructions = new_list
    return n


def _host_prep(h, p_L, A, B):
    """Shard tokens across cores; pre-transpose + pre-tile h; compute the
    top-2 gate matrix G on the host."""
    import ml_dtypes

    BF16 = ml_dtypes.bfloat16

    # hT[core][p, g, k, t] = h[core][token g*GROUP+t, d = k*128+p]
    h5 = np.asarray(h, dtype=np.float32).reshape(N_CORES, N_GROUPS, GROUP, KD, 128)
    hT = np.ascontiguousarray(h5.transpose(0, 4, 1, 3, 2)).astype(BF16)
    hT = hT.reshape(N_CORES, 128, N_GROUPS * KD * GROUP)

    # top-2 gates, f32-exact selection (matches jax.lax.top_k on distinct
    # values); G[core][c, t] = gates[t, c//16]
    p_flat = np.asarray(p_L, dtype=np.float32).reshape(T_FULL, E)
    thr = np.partition(p_flat, E - 2, axis=1)[:, E - 2 : E - 1]  # 2nd largest
    gates = np.where(p_flat >= thr, p_flat, np.float32(0.0))
    gt8 = gates.T.astype(BF16)  # [E, T_FULL]
    gt8 = np.ascontiguousarray(gt8.reshape(E, N_CORES, T_CORE).transpose(1, 0, 2))
    Mexp = np.zeros((E, C), dtype=np.float32)
    for e in range(E):
        Mexp[e, e * R : (e + 1) * R] = 1.0
    Mexp = Mexp.astype(BF16)

    # A_cat[d, c] = SCALING * A[e, r, d], pre-arranged [p, k*C + c]
    A_cat = (np.asarray(A, dtype=np.float32) * SCALING).transpose(2, 0, 1).reshape(D, C)
    A_arr = np.ascontiguousarray(
        A_cat.reshape(KD, 128, C).transpose(1, 0, 2).reshape(128, KD * C)
    ).astype(BF16)
    # B_cat[c, d] = B[e, d, r]
    B_cat = (
        np.asarray(B, dtype=np.float32).transpose(0, 2, 1).reshape(C, D).astype(BF16)
    )

    in_maps = []
    for i in range(N_CORES):
        in_maps.append(
            {
                "hT": hT[i],
                "gt8": gt8[i],
                "Mexp": Mexp,
                "A_cat": A_arr,
                "B_cat": B_cat,
            }
        )
    return in_maps


def _get_nc():
    if "nc" not in _CACHE:
        _CACHE["nc"] = _build_nc()
    return _CACHE["nc"]


def kernel(h, p_L, A, B):
    from concourse.bass_utils import run_bass_kernel_spmd

    nc = _get_nc()
    in_maps = _host_prep(h, p_L, A, B)
    res = run_bass_kernel_spmd(nc, in_maps, core_ids=list(range(N_CORES)))
    out = np.concatenate(
        [np.asarray(res.results[i]["out"]) for i in range(N_CORES)], axis=0
    )
    return out.astype(np.float32).reshape(B_SZ, S_SZ, D)


# revision 54
# speedup vs baseline: 1.1535x; 1.0159x over previous
"""Trainium2 Bass kernel for nn_LoRAPool (MoE top-2 LoRA expert pool).

Math (reference):
    gates[t,e] = p_L[t,e] if e in top-2 of p_L[t,:] else 0
    hr[t,e,r]  = sum_d h[t,d] * A[e,r,d]
    out[t,d]   = sum_{e,r} hr[t,e,r] * 2.0 * gates[t,e] * B[e,d,r]

Folded into two dense matmuls over c = (e,r) in [0,128):
    A_cat[d,c] = 2.0 * A[e,r,d];  B_cat[c,d] = B[e,d,r]
    U^T[c,t]   = sum_d A_cat[d,c] hT[d,t]       (stage 1, PE, bf16)
    Us[c,t]    = U^T[c,t] * G[c,t]              (gating, DVE)
    out[t,d]   = sum_c Us[c,t] B_cat[c,d]       (stage 2, PE, bf16)

Memory-bound: all large traffic (h in, out) is bf16 (tolerance 2e-2; bf16
end-to-end error is ~6e-3). h is pre-transposed AND pre-tiled on the host
([p, group, k, t] layout) so every device DMA has 8KB contiguous lines and
no on-device transposes are needed. The top-2 routing gates are computed
on the host (f32-exact selection, bf16 values) and streamed in compact
[8, tokens] form (32 KB/core); one tiny matmul per group expands them to
the dense mask G[c,t] = gates[t, c//16] during otherwise-idle PE time.
Token groups of 512 pipeline: group g's store overlaps group g+1's load.

Sharding: tokens (4*4096 = 16384) split evenly across 8 cores; weights
replicated.
"""

import numpy as np

N_CORES = 8
B_SZ, S_SZ, D = 4, 4096, 2048
E, R, C = 8, 16, 128
T_FULL = B_SZ * S_SZ            # 16384 tokens
T_CORE = T_FULL // N_CORES      # 2048 tokens per core
GROUP = 512                     # token group (stage-1 PSUM bank width)
N_GROUPS = T_CORE // GROUP      # 4
N_SUBTOT = T_CORE // 128        # 16 sub-blocks of 128 tokens per core
SUB_PER_GROUP = GROUP // 128    # 4
KD = D // 128                   # 16 contraction chunks
KH = KD // 2                    # chunks per hT half-DMA
SCALING = 2.0

_CACHE = {}


def _build_nc(split_waits=True):
    import concourse.bass as bass
    import concourse.tile as tile
    import concourse.mybir as mybir
    from contextlib import ExitStack

    f32 = mybir.dt.float32
    bf16 = mybir.dt.bfloat16

    nc = bass.Bass()
    # hT[p, g*KD*GROUP + k*GROUP + t] = h[token g*GROUP+t, d = k*128+p]
    ht_d = nc.declare_dram_parameter(
        "hT", [128, N_GROUPS * KD * GROUP], bf16, isOutput=False
    )
    gt_d = nc.declare_dram_parameter("gt8", [E, T_CORE], bf16, isOutput=False)
    m_d = nc.declare_dram_parameter("Mexp", [E, C], bf16, isOutput=False)
    a_d = nc.declare_dram_parameter("A_cat", [128, KD * C], bf16, isOutput=False)
    b_d = nc.declare_dram_parameter("B_cat", [C, D], bf16, isOutput=False)
    o_d = nc.declare_dram_parameter("out", [T_CORE, D], bf16, isOutput=True)

    OP = mybir.AluOpType

    with ExitStack() as ctx:
        tc = ctx.enter_context(tile.TileContext(nc))
        consts = ctx.enter_context(tc.tile_pool(name="consts", bufs=1))
        hpool = ctx.enter_context(tc.tile_pool(name="h", bufs=2 * N_GROUPS))
        gpool = ctx.enter_context(tc.tile_pool(name="gsb", bufs=N_GROUPS))
        utspool = ctx.enter_context(tc.tile_pool(name="uts", bufs=2))
        # one o_sb per sub-tile: slot reuse would make copies wait on the
        # store DMA queue counter (= ALL earlier DMAs incl. the h stream)
        opool = ctx.enter_context(tc.tile_pool(name="osb", bufs=N_SUBTOT))
        ps_u = ctx.enter_context(tc.tile_pool(name="ps_u", bufs=2, space="PSUM"))
        ps_o = ctx.enter_context(tc.tile_pool(name="ps_o", bufs=3, space="PSUM"))

        # ---- constants first (small, clears the queue) ----
        A_sb = consts.tile([128, KD * C], bf16)
        nc.sync.dma_start(out=A_sb, in_=a_d[:, :])
        gt_sb = consts.tile([E, T_CORE], bf16)
        nc.sync.dma_start(out=gt_sb, in_=gt_d[:, :])
        M_sb = consts.tile([E, C], bf16)
        nc.sync.dma_start(out=M_sb, in_=m_d[:, :])

        def issue_ht(g):
            tiles = []
            for h2 in range(2):
                ht = hpool.tile([128, KH, GROUP], bf16, tag="h", name=f"ht{g}_{h2}")
                off = g * KD * GROUP + h2 * KH * GROUP
                nc.sync.dma_start(
                    out=ht,
                    in_=ht_d[:, off : off + KH * GROUP].rearrange(
                        "p (k t) -> p k t", k=KH
                    ),
                )
                tiles.append(ht)
            return tiles

        # issue ALL h loads before any output store enters the (in-order)
        # sync queue — otherwise stores head-of-line block later h streams
        ht_tiles = {0: issue_ht(0)}
        B_sb = consts.tile([C, D], bf16)
        nc.sync.dma_start(out=B_sb, in_=b_d[:, :])
        for g in range(1, N_GROUPS):
            ht_tiles[g] = issue_ht(g)

        def stage1_chunk(g, U_ps, k0, k1):
            for k in range(k0, k1):
                nc.tensor.matmul(
                    U_ps,
                    lhsT=A_sb[:, k * C : (k + 1) * C],
                    rhs=ht_tiles[g][k // KH][:, k % KH, :],
                    start=(k == 0),
                    stop=(k == KD - 1),
                )

        def stage1(g):
            U_ps = ps_u.tile([128, GROUP], f32, tag="u", name=f"U{g}")
            stage1_chunk(g, U_ps, 0, KD)
            return U_ps

        copy_flip = [0]

        def stage2_subs(g, uts, subs):
            for s4 in subs:
                s = g * SUB_PER_GROUP + s4
                o_sb = opool.tile([128, D], bf16, tag="osb", name=f"osb{s}")
                for jh in range(2):
                    o_ps = ps_o.tile([128, 1024], f32, tag="o", name=f"o{s}_{jh}")
                    for j2 in range(2):
                        j = jh * 2 + j2
                        nc.tensor.matmul(
                            o_ps[:, j2 * 512 : (j2 + 1) * 512],
                            lhsT=uts[:, s4 * 128 : (s4 + 1) * 128],
                            rhs=B_sb[:, j * 512 : (j + 1) * 512],
                            start=True,
                            stop=True,
                        )
                    if s == N_SUBTOT - 1:
                        # final sub-tile is the exposed tail: split each
                        # copy across both engines to halve its latency
                        nc.vector.tensor_copy(
                            out=o_sb[:, jh * 1024 : jh * 1024 + 512],
                            in_=o_ps[:, :512],
                        )
                        nc.scalar.copy(
                            out=o_sb[:, jh * 1024 + 512 : (jh + 1) * 1024],
                            in_=o_ps[:, 512:],
                        )
                    elif s in (1, 5, 9) or jh == 1:
                        # bias copies toward the faster, half-idle ACT
                        # (12/18 DVE/ACT evens both engines at ~17-18us;
                        # ACT-ACT pairs only on mid-group subs)
                        nc.scalar.copy(
                            out=o_sb[:, jh * 1024 : (jh + 1) * 1024], in_=o_ps
                        )
                    else:
                        nc.vector.tensor_copy(
                            out=o_sb[:, jh * 1024 : (jh + 1) * 1024], in_=o_ps
                        )
                    copy_flip[0] += 1
                    # store each half right after its copy: halves the
                    # copy->store latency on every group-boundary chain
                    nc.sync.dma_start(
                        out=o_d[s * 128 : (s + 1) * 128, jh * 1024 : (jh + 1) * 1024],
                        in_=o_sb[:, jh * 1024 : (jh + 1) * 1024],
                    )

        # expand gt[8, t] -> dense G[c, t] per group up front. Expansion
        # tiles live in ps_u (same shape as U, no effect on the out-pool
        # slot rotation); copies go to the early-idle DVE.
        G_list = []
        for g in range(N_GROUPS):
            G_ps = ps_u.tile([128, GROUP], f32, tag="u", name=f"Gps{g}")
            nc.tensor.matmul(
                G_ps,
                lhsT=M_sb,
                rhs=gt_sb[:, g * GROUP : (g + 1) * GROUP],
                start=True,
                stop=True,
            )
            G_sbg = gpool.tile([128, GROUP], bf16, tag="gsb", name=f"Gsb{g}")
            nc.vector.tensor_copy(out=G_sbg, in_=G_ps)
            G_list.append(G_sbg)

        # Monotone logical waits stop the scheduler from hoisting group g+1
        # work above group g's store pipeline. Stage-1 of group g+1 is
        # emitted in 4-matmul chunks between group g's stage-2 sub-tiles so
        # scheduling mispredictions cost at most one small chunk.
        U_cur = stage1(0)
        for g in range(N_GROUPS):
            tc.tile_set_cur_wait(g + 1)
            uts = utspool.tile([128, GROUP], bf16, tag="uts", name=f"uts{g}")
            nc.vector.tensor_tensor(
                out=uts, in0=U_cur, in1=G_list[g], op=OP.mult
            )
            U_next = None
            if g + 1 < N_GROUPS:
                U_next = ps_u.tile([128, GROUP], f32, tag="u", name=f"U{g + 1}")
            for s4 in range(SUB_PER_GROUP):
                stage2_subs(g, uts, (s4,))
                if U_next is not None:
                    stage1_chunk(g + 1, U_next, 4 * s4, 4 * s4 + 4)
            U_cur = U_next

    if split_waits:
        _split_matmul_waits(nc)
    return nc


def _split_matmul_waits(nc, max_waits=1):
    """Walrus codegen allows only one sync-wait slot on self-loading
    Matmult instructions. Move surplus waits onto a no-op EventSemaphore
    inserted immediately before, same engine — identical semantics."""
    import concourse.mybir as mybir

    n = 0
    for f in nc.m.functions:
        for blk in f.blocks:
            insts = blk.instructions
            new_list = []
            changed = False
            for inst in insts:
                si = inst.sync_info
                if (
                    type(inst).__name__ != "InstEventSemaphore"
                    and si is not None
                    and si.on_wait
                    and len(si.on_wait) > max_waits
                ):
                    surplus = list(si.on_wait[:-max_waits])
                    keep = list(si.on_wait[-max_waits:])
                    for i in range(0, len(surplus), 2):
                        n += 1
                        ev = mybir.InstEventSemaphore(
                            name=f"I-swsplit-{n}", ins=[], outs=[]
                        )
                        ev.engine = inst.engine
                        ev.sync_info = mybir.SyncInfo(
                            on_wait=surplus[i : i + 2], on_update=[]
                        )
                        new_list.append(ev)
                    inst.sync_info = mybir.SyncInfo(
                        on_wait=keep, on_update=list(si.on_update or [])
                    )
                    changed = True
                new_list.append(inst)
            if changed:
                blk.instructions = new_list
    return n


def _host_prep(h, p_L, A, B):
    """Shard tokens across cores; pre-transpose + pre-tile h; compute the
    top-2 gate matrix G on the host."""
    import ml_dtypes

    BF16 = ml_dtypes.bfloat16

    # hT[core][p, g, k, t] = h[core][token g*GROUP+t, d = k*128+p]
    h5 = np.asarray(h, dtype=np.float32).reshape(N_CORES, N_GROUPS, GROUP, KD, 128)
    hT = np.ascontiguousarray(h5.transpose(0, 4, 1, 3, 2)).astype(BF16)
    hT = hT.reshape(N_CORES, 128, N_GROUPS * KD * GROUP)

    # top-2 gates, f32-exact selection (matches jax.lax.top_k on distinct
    # values); G[core][c, t] = gates[t, c//16]
    p_flat = np.asarray(p_L, dtype=np.float32).reshape(T_FULL, E)
    thr = np.partition(p_flat, E - 2, axis=1)[:, E - 2 : E - 1]  # 2nd largest
    gates = np.where(p_flat >= thr, p_flat, np.float32(0.0))
    gt8 = gates.T.astype(BF16)  # [E, T_FULL]
    gt8 = np.ascontiguousarray(gt8.reshape(E, N_CORES, T_CORE).transpose(1, 0, 2))
    Mexp = np.zeros((E, C), dtype=np.float32)
    for e in range(E):
        Mexp[e, e * R : (e + 1) * R] = 1.0
    Mexp = Mexp.astype(BF16)

    # A_cat[d, c] = SCALING * A[e, r, d], pre-arranged [p, k*C + c]
    A_cat = (np.asarray(A, dtype=np.float32) * SCALING).transpose(2, 0, 1).reshape(D, C)
    A_arr = np.ascontiguousarray(
        A_cat.reshape(KD, 128, C).transpose(1, 0, 2).reshape(128, KD * C)
    ).astype(BF16)
    # B_cat[c, d] = B[e, d, r]
    B_cat = (
        np.asarray(B, dtype=np.float32).transpose(0, 2, 1).reshape(C, D).astype(BF16)
    )

    in_maps = []
    for i in range(N_CORES):
        in_maps.append(
            {
                "hT": hT[i],
                "gt8": gt8[i],
                "Mexp": Mexp,
                "A_cat": A_arr,
                "B_cat": B_cat,
            }
        )
    return in_maps


def _get_nc():
    if "nc" not in _CACHE:
        _CACHE["nc"] = _build_nc()
    return _CACHE["nc"]


def kernel(h, p_L, A, B):
    from concourse.bass_utils import run_bass_kernel_spmd

    nc = _get_nc()
    in_maps = _host_prep(h, p_L, A, B)
    res = run_bass_kernel_spmd(nc, in_maps, core_ids=list(range(N_CORES)))
    out = np.concatenate(
        [np.asarray(res.results[i]["out"]) for i in range(N_CORES)], axis=0
    )
    return out.astype(np.float32).reshape(B_SZ, S_SZ, D)
